# revision 22
# baseline (speedup 1.0000x reference)
"""Trainium2 Bass kernel for nn_AlignmentEncoder.

Reference computation (per batch b):
    keys    += (g @ w_kspk.T + b_kspk)[:, None]          # [Ct, T2]
    queries += (g @ w_qspk.T + b_qspk)[:, None]          # [Cs, T1]
    k_enc = conv1(relu(conv3(keys, wk1, bk1)), wk2, bk2)       # [Ca, T2]
    q_enc = conv1(relu(conv1(relu(conv3(queries, wq1, bq1)), wq2, bq2)), wq3, bq3)
    logits[t, s] = -TEMP * (|q|^2[t] + |k|^2[s] - 2 q.k[t, s])
    attn_logprob = log_softmax(logits, axis=s) + log(prior + 1e-8)
    attn = softmax(attn_logprob + (mask == 0 ? -inf : 0), axis=s)

Device-side algebra:
  * log_softmax and the final softmax are invariant to per-row (t)
    constants, so the |q|^2 term cancels and is never computed.
  * logits come from one augmented matmul:
        lhsT = [2*TEMP*q_enc ; 1], rhs = [k_enc ; -TEMP*|k|^2]
  * attn = normalize(exp(lp_out)) where lp_out = logits + log(prior+1e-8)
    + nlse and nlse = -logsumexp(logits): the row shift cancels.

Scheduling structure (from trace analysis):
  * ACT table loads cost ~2.7us per function-set switch -> only Relu /
    Exp / Ln ever run on ScalarE, batched by phase; all Identity /
    Square / Copy evacuations run on VectorE.
  * T1 is tiled as t = 8p + j (p in [0,125), j in [0,8)) so the prior
    load and both output stores are single big DMAs with 6.4KB
    contiguous runs per partition.
  * All weights/inputs are host-prepped into partition-major layouts so
    every DMA descriptor is a multi-KB contiguous run.

Sharding: pure data parallel, batch 16 -> 8 cores x 2 batches.
"""

import sys

for _p in ("/opt/trn_rl_repo",):
    if _p not in sys.path:
        sys.path.insert(0, _p)

import numpy as np
import ml_dtypes

import concourse.bass as bass
import concourse.bacc as bacc
import concourse.tile as tile
from concourse import mybir
from concourse.bass_utils import run_bass_kernel_spmd
from concourse.compiler_utils import get_compiler_flags, set_compiler_flags

# the default backend options disable walrus's redundant-LDWEIGHTS elision;
# this kernel is PE-bound with a 1:1 LDW:matmul ratio, so turn it back on
set_compiler_flags([
    f.replace("--enable-ldw-opt=false", "--enable-ldw-opt=true")
    for f in get_compiler_flags()
])

F32 = mybir.dt.float32
BF16 = mybir.dt.bfloat16
I32 = mybir.dt.int32
AF = mybir.ActivationFunctionType
ALU = mybir.AluOpType

TEMP = 0.0005
B2 = 2          # batches per core
CT, T2 = 512, 200
CS, T1 = 80, 1000
CA = 80
G = 256
TP = 125        # t-partition count;  t = 8*p + j
NJ = 8          # j-chunks


def build_nc(use_mask: bool = False) -> bass.Bass:
    nc = bacc.Bacc()

    keys_d = nc.dram_tensor("keys", [128, 4, B2, T2], BF16, kind="ExternalInput")
    qry_d = nc.dram_tensor("queries", [CS, B2, T1], BF16, kind="ExternalInput")
    pri_d = nc.dram_tensor("prior", [B2, T1, T2], BF16, kind="ExternalInput")
    g_d = nc.dram_tensor("g", [128, 2, B2], BF16, kind="ExternalInput")
    w1t_d = nc.dram_tensor("w1t", [8, 128, 3, 4, 128], BF16, kind="ExternalInput")
    w2t_d = nc.dram_tensor("w2t", [128, 8, 80], BF16, kind="ExternalInput")
    q1t_d = nc.dram_tensor("q1t", [80, 3, 160], BF16, kind="ExternalInput")
    q2t_d = nc.dram_tensor("q2t", [80, 2, 80], BF16, kind="ExternalInput")
    q3t_d = nc.dram_tensor("q3t", [80, 80], BF16, kind="ExternalInput")
    kspkt_d = nc.dram_tensor("kspkt", [128, 2, 512], BF16, kind="ExternalInput")
    qspkt_d = nc.dram_tensor("qspkt", [128, 2, 80], BF16, kind="ExternalInput")
    bk1_d = nc.dram_tensor("bk1", [128, 8], F32, kind="ExternalInput")
    bk2_d = nc.dram_tensor("bk2", [80, 1], F32, kind="ExternalInput")
    bq1_d = nc.dram_tensor("bq1", [80, 2], F32, kind="ExternalInput")
    bq2_d = nc.dram_tensor("bq2", [80, 1], F32, kind="ExternalInput")
    bq3_d = nc.dram_tensor("bq3", [80, 1], F32, kind="ExternalInput")
    bkspk_d = nc.dram_tensor("bkspk", [128, 4], F32, kind="ExternalInput")
    bqspk_d = nc.dram_tensor("bqspk", [80, 1], F32, kind="ExternalInput")
    if use_mask:
        mask_d = nc.dram_tensor("mask", [B2, T2], I32, kind="ExternalInput")

    attn_d = nc.dram_tensor("attn", [B2, T1, T2], BF16, kind="ExternalOutput")
    alp_d = nc.dram_tensor("alp", [B2, T1, T2], BF16, kind="ExternalOutput")
    # [p, j, s] views of the t-major DRAM tensors (t = 8p + j)
    pri_v = [pri_d[b].rearrange("(p j) s -> p j s", j=NJ) for b in range(B2)]
    attn_v = [attn_d[b].rearrange("(p j) s -> p j s", j=NJ) for b in range(B2)]
    alp_v = [alp_d[b].rearrange("(p j) s -> p j s", j=NJ) for b in range(B2)]

    with tile.TileContext(nc) as tc:
        with (
            tc.tile_pool(name="consts", bufs=1) as consts,
            tc.tile_pool(name="persist", bufs=1) as persist,
            tc.tile_pool(name="work", bufs=2) as work,
            tc.tile_pool(name="psA", bufs=2, space="PSUM") as psA,
            tc.tile_pool(name="psQ", bufs=2, space="PSUM") as psQ,
            tc.tile_pool(name="psS", bufs=3, space="PSUM") as psS,
            tc.tile_pool(name="psT", bufs=1, space="PSUM") as psT,
        ):
            # ---------------- input loads (sync ring is FIFO: kconv chain first) ----------------
            kspkt = consts.tile([128, 2, 512], BF16)
            nc.sync.dma_start(kspkt, kspkt_d[:])
            gT = consts.tile([128, 2, B2], BF16)
            nc.sync.dma_start(gT, g_d[:])
            bkspk = consts.tile([128, 4], F32)
            nc.sync.dma_start(bkspk, bkspk_d[:])
            bk1 = consts.tile([128, 8], F32)
            nc.sync.dma_start(bk1, bk1_d[:])
            keysf = work.tile([128, 4, B2, T2], BF16, tag="keysf")
            nc.sync.dma_start(keysf, keys_d[:])
            w1t = consts.tile([128, 8, 3, 4, 128], BF16)   # [p, co, k, c, m]
            for co in range(0, 8, 2):
                nc.sync.dma_start(w1t[:, co], w1t_d[co])
            w2t = consts.tile([128, 8, 80], BF16)
            nc.sync.dma_start(w2t, w2t_d[:])
            bk2 = consts.tile([80, 1], F32)
            nc.sync.dma_start(bk2, bk2_d[:])

            # ---------------- q-chain loads (scalar ring: smalls first) ----------------
            q1t = consts.tile([80, 3, 160], BF16)
            nc.scalar.dma_start(q1t, q1t_d[:])
            q2t = consts.tile([80, 2, 80], BF16)
            nc.scalar.dma_start(q2t, q2t_d[:])
            q3t = consts.tile([80, 80], BF16)
            nc.scalar.dma_start(q3t, q3t_d[:])
            qspkt = consts.tile([128, 2, 80], BF16)
            nc.scalar.dma_start(qspkt, qspkt_d[:])
            bq1 = consts.tile([80, 2], F32)
            nc.scalar.dma_start(bq1, bq1_d[:])
            bq2 = consts.tile([80, 1], F32)
            nc.scalar.dma_start(bq2, bq2_d[:])
            bq3 = consts.tile([80, 1], F32)
            nc.scalar.dma_start(bq3, bq3_d[:])
            bqspk = consts.tile([80, 1], F32)
            nc.scalar.dma_start(bqspk, bqspk_d[:])
            qf = work.tile([80, B2, T1], BF16, tag="qf")
            nc.scalar.dma_start(qf, qry_d[:])
            for co in range(1, 8, 2):
                nc.scalar.dma_start(w1t[:, co], w1t_d[co])

            # ---------------- priors (gpsimd ring) + logpri (ACT: Ln) ----------------
            eps_b = consts.tile([128, 1], F32)
            nc.gpsimd.memset(eps_b, 1e-8)
            ones80 = consts.tile([80, 1], F32)
            nc.gpsimd.memset(ones80, 1.0)
            pri_sb = []
            logpri = []
            for b in range(B2):
                pt = work.tile([TP, NJ, T2], BF16, tag="pri", name=f"pri{b}")
                nc.sync.dma_start(pt, pri_v[b])
                pri_sb.append(pt)
            for b in range(B2):
                lg = persist.tile([TP, NJ, T2], F32, name=f"logpri{b}")
                nc.scalar.activation(lg, pri_sb[b], AF.Ln,
                                     bias=eps_b[:TP, 0:1])
                logpri.append(lg)

            # ---------------- speaker projections (evac on DVE) ----------------
            kspk = persist.tile([128, 4, B2], F32)
            for c in range(4):
                ps = psT.tile([128, B2], F32, tag="tiny")
                for o in range(2):
                    nc.tensor.matmul(
                        ps, kspkt[:, o, c * 128:(c + 1) * 128], gT[:, o, :],
                        start=(o == 0), stop=(o == 1),
                    )
                nc.vector.tensor_scalar_add(kspk[:, c, :], ps, bkspk[:, c:c + 1])
            qspk = persist.tile([80, B2], F32)
            psq = psT.tile([80, B2], F32, tag="tiny")
            for o in range(2):
                nc.tensor.matmul(psq, qspkt[:, o, :], gT[:, o, :],
                                 start=(o == 0), stop=(o == 1))
            nc.vector.tensor_scalar_add(qspk, psq, bqspk[:, 0:1])

            # ---------------- conditioned + padded inputs ----------------
            keys_bf = persist.tile([128, 4, B2, T2 + 2], BF16)
            nc.gpsimd.memset(keys_bf[:, :, :, 0:1], 0.0)
            nc.gpsimd.memset(keys_bf[:, :, :, T2 + 1:T2 + 2], 0.0)
            for c in range(4):
                for b in range(B2):
                    nc.vector.tensor_scalar_add(
                        keys_bf[:, c, b, 1:T2 + 1], keysf[:, c, b, :],
                        kspk[:, c, b:b + 1],
                    )
            q_bf = persist.tile([80, B2, T1 + 2], BF16)
            nc.gpsimd.memset(q_bf[:, :, 0:1], 0.0)
            nc.gpsimd.memset(q_bf[:, :, T1 + 1:T1 + 2], 0.0)
            for b in range(B2):
                nc.vector.tensor_scalar_add(
                    q_bf[:, b, 1:T1 + 1], qf[:, b, :], qspk[:, b:b + 1]
                )

            # ---------------- query convs first: PE work that needs no w1t ----------------
            x1 = persist.tile([80, 2, B2, 2, 500], BF16)  # [p, m, b, half, t]
            for m in range(2):
                for b in range(B2):
                    for j in range(2):
                        ps = psQ.tile([80, 500], F32, tag="q")
                        for k in range(3):
                            nc.tensor.matmul(
                                ps,
                                q1t[:, k, m * 80:(m + 1) * 80],
                                q_bf[:, b, j * 500 + k:j * 500 + k + 500],
                                start=(k == 0), stop=(k == 2),
                            )
                        nc.scalar.activation(x1[:, m, b, j], ps, AF.Relu,
                                             bias=bq1[:, m:m + 1])
            x2q = persist.tile([80, B2, 2, 500], BF16)
            for b in range(B2):
                for j in range(2):
                    ps = psQ.tile([80, 500], F32, tag="q")
                    for ci in range(2):
                        nc.tensor.matmul(ps, q2t[:, ci, :], x1[:, ci, b, j],
                                         start=(ci == 0), stop=(ci == 1))
                    nc.scalar.activation(x2q[:, b, j], ps, AF.Relu,
                                         bias=bq2[:, 0:1])
            q_aug = persist.tile([97, B2, T1], BF16)
            nc.gpsimd.memset(q_aug, 0.0)
            nc.gpsimd.memset(q_aug[96:97], 1.0)
            for b in range(B2):
                for j in range(2):
                    ps = psQ.tile([80, 500], F32, tag="q")
                    nc.tensor.matmul(ps, q3t, x2q[:, b, j], start=True, stop=True)
                    nc.vector.tensor_scalar_add(
                        q_aug[0:80, b, j * 500:(j + 1) * 500], ps, bq3[:, 0:1])
            # [p(contraction), b, j, tp] view: column index tp maps to t = 8*tp + j
            q_aug_v = q_aug.rearrange("p b (tp j) -> p b j tp", j=NJ)

            # ---------------- key conv1 (k=3) + relu (ACT) -> x2, kconv2 interleaved ----------------
            x2 = persist.tile([128, 8, B2, T2], BF16)
            rhs_sc = persist.tile([97, B2, T2], BF16)
            nc.gpsimd.memset(rhs_sc, 0.0)
            ps2 = psQ.tile([80, B2, T2], F32, tag="q")
            for co in range(8):
                ps = psA.tile([128, B2, T2], F32, tag="kc1")
                for k in range(3):
                    for c in range(4):
                        nc.tensor.matmul(
                            ps,
                            w1t[:, co, k, c, :],
                            keys_bf[:, c, :, k:k + T2],
                            start=(c == 0 and k == 0),
                            stop=(c == 3 and k == 2),
                        )
                nc.scalar.activation(x2[:, co], ps, AF.Relu, bias=bk1[:, co:co + 1])
                nc.tensor.matmul(ps2, w2t[:, co, :], x2[:, co],
                                 start=(co == 0), stop=(co == 7))
            # k_enc rows (bf16) and |k|^2 row, straight from the conv2 PSUM
            nc.vector.tensor_scalar_add(rhs_sc[0:80], ps2, bk2[:, 0:1])
            ksq = work.tile([80, B2, T2], F32, tag="ksq")
            nc.scalar.activation(ksq, ps2, AF.Square, bias=bk2[:, 0:1])
            ps3 = psT.tile([1, B2, T2], F32, tag="tiny")
            nc.tensor.matmul(ps3, ones80, ksq, start=True, stop=True)
            nc.vector.tensor_scalar_mul(rhs_sc[96:97], ps3, -TEMP)

            # ---------------- mask replication (generic path only) ----------------
            if use_mask:
                mask_sb = consts.tile([1, B2, T2], I32)
                nc.sync.dma_start(
                    mask_sb,
                    mask_d.rearrange("b t -> (b t)")[None, :]
                          .rearrange("o (b t) -> o b t", b=B2))
                m01 = consts.tile([1, B2, T2], F32)
                nc.vector.tensor_scalar(m01, mask_sb, 0, None, ALU.not_equal)
                ones1 = consts.tile([1, 128], F32)
                nc.vector.memset(ones1, 1.0)
                psm = psT.tile([128, B2, T2], F32, tag="tiny")
                for b in range(B2):
                    nc.tensor.matmul(psm[:, b, :], ones1, m01[:, b, :],
                                     start=True, stop=True)
                mrep = persist.tile([128, B2, T2], F32)
                nc.vector.tensor_copy(mrep, psm)

            # ---------------- scores + softmax, pipelined per batch ----------------
            # attn = normalize_s(exp(lp_pre)): the -lse row shift cancels in
            # the softmax, so the attn branch never waits on logsumexp.
            NP = NJ // 2  # j-pairs (each pair shares one PSUM bank)
            rsums = persist.tile([TP, B2, NJ], F32)
            rinvs = persist.tile([TP, B2, NJ], F32)
            nlse = persist.tile([TP, B2, NJ], F32)
            fsums = persist.tile([TP, B2, NJ], F32)
            frs = persist.tile([TP, B2, NJ], F32)
            out_eng = [nc.sync, nc.gpsimd]
            e_sb = []
            lp_sb = []
            for b in range(B2):
                e_m = persist.tile([TP, NJ, T2], F32, name=f"e{b}")
                lp_m = persist.tile([TP, NJ, T2], F32, name=f"lp{b}")
                e_sb.append(e_m)
                lp_sb.append(lp_m)
                for jp in range(NP):
                    ps = psS.tile([TP, 2, T2], F32, tag="s")
                    for u in range(2):
                        j = jp * 2 + u
                        nc.tensor.matmul(ps[:, u, :], q_aug_v[:, b, j, :],
                                         rhs_sc[:, b, :], start=True, stop=True)
                    pr = slice(jp * 2, jp * 2 + 2)
                    for u in range(2):
                        j = jp * 2 + u
                        nc.scalar.activation(e_m[:, j, :], ps[:, u, :], AF.Exp,
                                             accum_out=rsums[:, b, j:j + 1])
                        nc.vector.tensor_tensor(
                            lp_m[:, j, :], ps[:, u, :], logpri[b][:, j, :],
                            ALU.add)
                    if use_mask:
                        # f = exp(lp_pre) * mask, fsum via DVE
                        nc.scalar.activation(e_m[:, pr, :], lp_m[:, pr, :],
                                             AF.Exp)
                        nc.vector.tensor_tensor(
                            e_m[:, pr, :], e_m[:, pr, :],
                            mrep[:TP, b, None, :].to_broadcast([TP, 2, T2]),
                            ALU.mult)
                        nc.vector.reduce_sum(fsums[:, b, pr], e_m[:, pr, :],
                                             axis=mybir.AxisListType.X)
                    else:
                        nc.scalar.activation(e_m[:, pr, :], lp_m[:, pr, :],
                                             AF.Exp)
                        nc.vector.reduce_sum(fsums[:, b, pr], e_m[:, pr, :],
                                             axis=mybir.AxisListType.X)
                # attn = f / fsum (bf16 out), out as soon as each pair is scaled
                at_m = persist.tile([TP, NJ, T2], BF16, name=f"at{b}")
                nc.vector.reciprocal(frs[:, b, :], fsums[:, b, :])
                for jp in range(NP):
                    pr = slice(jp * 2, jp * 2 + 2)
                    for u in range(2):
                        j = jp * 2 + u
                        nc.vector.tensor_scalar_mul(
                            at_m[:, j, :], e_m[:, j, :], frs[:, b, j:j + 1])
                    out_eng[jp % 2].dma_start(attn_v[b][:, pr, :],
                                              at_m[:, pr, :])

            # attn_logprob needs nlse = ln(1/rowsum): one Ln for all rows
            nc.vector.reciprocal(rinvs, rsums)
            nc.scalar.activation(nlse, rinvs, AF.Ln)
            for b in range(B2):
                lo_m = persist.tile([TP, NJ, T2], BF16, name=f"lo{b}")
                for jp in range(NP):
                    pr = slice(jp * 2, jp * 2 + 2)
                    for u in range(2):
                        j = jp * 2 + u
                        nc.vector.tensor_scalar_add(
                            lo_m[:, j, :], lp_sb[b][:, j, :],
                            nlse[:, b, j:j + 1])
                    out_eng[(jp + 1) % 2].dma_start(alp_v[b][:, pr, :],
                                                    lo_m[:, pr, :])

    nc.compile()
    return nc


_CACHE: dict = {}


def _prep_shared(wk1, bk1, wk2, bk2, wq1, bq1, wq2, bq2, wq3, bq3,
                 w_kspk, b_kspk, w_qspk, b_qspk):
    bf = ml_dtypes.bfloat16
    f32 = np.float32

    def a(x):
        return np.ascontiguousarray(np.asarray(x, f32))

    def c(x):
        return np.ascontiguousarray(x)

    shared = {
        # [p(cin%128), k, cin//128, cout]
        # [co, p(cin%128), k, cin//128, m(cout%128)]
        "w1t": c(np.transpose(a(wk1), (2, 1, 0)).reshape(3, 4, 128, 8, 128)
                 .transpose(3, 2, 0, 1, 4)).astype(bf),
        "w2t": c(a(wk2)[:, :, 0].T.reshape(8, 128, 80)
                 .transpose(1, 0, 2)).astype(bf),
        # [cin(p)=80, k, cout=160]; wq1 is [160, 80, 3]
        "q1t": c(np.transpose(a(wq1), (1, 2, 0))).astype(bf),
        "q2t": c(a(wq2)[:, :, 0].T.reshape(2, 80, 80)
                 .transpose(1, 0, 2)).astype(bf),
        "q3t": c(2.0 * TEMP * a(wq3)[:, :, 0].T).astype(bf),
        "kspkt": c(a(w_kspk).T.reshape(2, 128, 512)
                   .transpose(1, 0, 2)).astype(bf),
        "qspkt": c(a(w_qspk).T.reshape(2, 128, 80)
                   .transpose(1, 0, 2)).astype(bf),
        "bk1": c(a(bk1).reshape(8, 128).T),
        "bk2": a(bk2).reshape(80, 1),
        "bq1": c(a(bq1).reshape(2, 80).T),
        "bq2": a(bq2).reshape(80, 1),
        "bq3": (2.0 * TEMP * a(bq3)).reshape(80, 1),
        "bkspk": c(a(b_kspk).reshape(4, 128).T),
        "bqspk": a(b_qspk).reshape(80, 1),
    }
    return shared




def make_in_maps(queries, keys, mask, attn_prior, g,
                 wk1, bk1, wk2, bk2, wq1, bq1, wq2, bq2, wq3, bq3,
                 w_kspk, b_kspk, w_qspk, b_qspk,
                 n_cores=8, use_mask=False):
    shared = _prep_shared(wk1, bk1, wk2, bk2, wq1, bq1, wq2, bq2, wq3, bq3,
                          w_kspk, b_kspk, w_qspk, b_qspk)
    queries = np.asarray(queries, np.float32)
    keys = np.asarray(keys, np.float32)
    attn_prior = np.asarray(attn_prior, np.float32)
    g = np.asarray(g, np.float32)
    mask = np.ascontiguousarray(np.asarray(mask, np.int32))
    in_maps = []
    for ci in range(n_cores):
        sl = slice(B2 * ci, B2 * (ci + 1))
        m = dict(shared)
        m["keys"] = np.ascontiguousarray(
            keys[sl].reshape(B2, 4, 128, T2).transpose(2, 1, 0, 3)
        ).astype(ml_dtypes.bfloat16)
        m["queries"] = np.ascontiguousarray(queries[sl].transpose(1, 0, 2)).astype(ml_dtypes.bfloat16)
        m["prior"] = np.ascontiguousarray(attn_prior[sl]).astype(ml_dtypes.bfloat16)
        m["g"] = np.ascontiguousarray(
            np.transpose(g[sl].reshape(B2, 2, 128), (2, 1, 0))
        ).astype(ml_dtypes.bfloat16)
        if use_mask:
            m["mask"] = mask[sl]
        in_maps.append(m)
    return in_maps

def kernel(queries, keys, mask, attn_prior, g,
           wk1, bk1, wk2, bk2, wq1, bq1, wq2, bq2, wq3, bq3,
           w_kspk, b_kspk, w_qspk, b_qspk,
           _trace=False, _trace_kwargs=None):
    n_cores = 8
    B = 16
    use_mask = not bool(np.all(np.asarray(mask) != 0))

    key = ("nc", use_mask)
    if key not in _CACHE:
        _CACHE[key] = build_nc(use_mask)
    nc = _CACHE[key]

    in_maps = make_in_maps(
        queries, keys, mask, attn_prior, g,
        wk1, bk1, wk2, bk2, wq1, bq1, wq2, bq2, wq3, bq3,
        w_kspk, b_kspk, w_qspk, b_qspk,
        n_cores=n_cores, use_mask=use_mask)

    kw = {}
    if _trace:
        kw["trace"] = True
        kw.update(_trace_kwargs or {})
    res = run_bass_kernel_spmd(nc, in_maps, list(range(n_cores)), **kw)

    attn = np.concatenate([r["attn"] for r in res.results], axis=0)
    alp = np.concatenate([r["alp"] for r in res.results], axis=0)
    attn = attn.reshape(B, 1, T1, T2).astype(np.float32)
    alp = alp.reshape(B, 1, T1, T2).astype(np.float32)
    if _trace:
        return (attn, alp), res
    return attn, alp


# revision 23
# speedup vs baseline: 1.1248x; 1.1248x over previous
"""Trainium2 Bass kernel for nn_AlignmentEncoder.

Reference computation (per batch b):
    keys    += (g @ w_kspk.T + b_kspk)[:, None]          # [Ct, T2]
    queries += (g @ w_qspk.T + b_qspk)[:, None]          # [Cs, T1]
    k_enc = conv1(relu(conv3(keys, wk1, bk1)), wk2, bk2)       # [Ca, T2]
    q_enc = conv1(relu(conv1(relu(conv3(queries, wq1, bq1)), wq2, bq2)), wq3, bq3)
    logits[t, s] = -TEMP * (|q|^2[t] + |k|^2[s] - 2 q.k[t, s])
    attn_logprob = log_softmax(logits, axis=s) + log(prior + 1e-8)
    attn = softmax(attn_logprob + (mask == 0 ? -inf : 0), axis=s)

Device-side algebra:
  * log_softmax and the final softmax are invariant to per-row (t)
    constants, so the |q|^2 term cancels and is never computed.
  * logits come from one augmented matmul:
        lhsT = [2*TEMP*q_enc ; 1], rhs = [k_enc ; -TEMP*|k|^2]
  * attn = normalize(exp(lp_out)) where lp_out = logits + log(prior+1e-8)
    + nlse and nlse = -logsumexp(logits): the row shift cancels.

Scheduling structure (from trace analysis):
  * ACT table loads cost ~2.7us per function-set switch -> only Relu /
    Exp / Ln ever run on ScalarE, batched by phase; all Identity /
    Square / Copy evacuations run on VectorE.
  * T1 is tiled as t = 8p + j (p in [0,125), j in [0,8)) so the prior
    load and both output stores are single big DMAs with 6.4KB
    contiguous runs per partition.
  * All weights/inputs are host-prepped into partition-major layouts so
    every DMA descriptor is a multi-KB contiguous run.

Sharding: pure data parallel, batch 16 -> 8 cores x 2 batches.
"""

import sys

for _p in ("/opt/trn_rl_repo",):
    if _p not in sys.path:
        sys.path.insert(0, _p)

import numpy as np
import ml_dtypes

import concourse.bass as bass
import concourse.bacc as bacc
import concourse.tile as tile
from concourse import mybir
from concourse.bass_utils import run_bass_kernel_spmd
from concourse.compiler_utils import get_compiler_flags, set_compiler_flags

# the default backend options disable walrus's redundant-LDWEIGHTS elision;
# this kernel is PE-bound with a 1:1 LDW:matmul ratio, so turn it back on
set_compiler_flags([
    f.replace("--enable-ldw-opt=false", "--enable-ldw-opt=true")
    for f in get_compiler_flags()
])

F32 = mybir.dt.float32
BF16 = mybir.dt.bfloat16
I32 = mybir.dt.int32
AF = mybir.ActivationFunctionType
ALU = mybir.AluOpType

TEMP = 0.0005
B2 = 2          # batches per core
CT, T2 = 512, 200
CS, T1 = 80, 1000
CA = 80
G = 256
TP = 125        # t-partition count;  t = 8*p + j
NJ = 8          # j-chunks


def build_nc(use_mask: bool = False) -> bass.Bass:
    nc = bacc.Bacc()

    keys_d = nc.dram_tensor("keys", [128, 4, B2, T2], BF16, kind="ExternalInput")
    qry_d = nc.dram_tensor("queries", [CS, B2, T1], BF16, kind="ExternalInput")
    pri_d = nc.dram_tensor("prior", [B2, T1, T2], BF16, kind="ExternalInput")
    g_d = nc.dram_tensor("g", [128, 2, B2], BF16, kind="ExternalInput")
    w1t_d = nc.dram_tensor("w1t", [8, 128, 3, 4, 128], BF16, kind="ExternalInput")
    w2t_d = nc.dram_tensor("w2t", [128, 8, 80], BF16, kind="ExternalInput")
    q1t_d = nc.dram_tensor("q1t", [80, 3, 160], BF16, kind="ExternalInput")
    q2t_d = nc.dram_tensor("q2t", [80, 2, 80], BF16, kind="ExternalInput")
    q3t_d = nc.dram_tensor("q3t", [80, 80], BF16, kind="ExternalInput")
    kspkt_d = nc.dram_tensor("kspkt", [128, 2, 512], BF16, kind="ExternalInput")
    qspkt_d = nc.dram_tensor("qspkt", [128, 2, 80], BF16, kind="ExternalInput")
    bk1_d = nc.dram_tensor("bk1", [128, 8], F32, kind="ExternalInput")
    bk2_d = nc.dram_tensor("bk2", [80, 1], F32, kind="ExternalInput")
    bq1_d = nc.dram_tensor("bq1", [80, 2], F32, kind="ExternalInput")
    bq2_d = nc.dram_tensor("bq2", [80, 1], F32, kind="ExternalInput")
    bq3_d = nc.dram_tensor("bq3", [80, 1], F32, kind="ExternalInput")
    bkspk_d = nc.dram_tensor("bkspk", [128, 4], F32, kind="ExternalInput")
    bqspk_d = nc.dram_tensor("bqspk", [80, 1], F32, kind="ExternalInput")
    if use_mask:
        mask_d = nc.dram_tensor("mask", [B2, T2], I32, kind="ExternalInput")

    attn_d = nc.dram_tensor("attn", [B2, T1, T2], BF16, kind="ExternalOutput")
    alp_d = nc.dram_tensor("alp", [B2, T1, T2], BF16, kind="ExternalOutput")
    # [p, j, s] views of the t-major DRAM tensors (t = 8p + j)
    pri_v = [pri_d[b].rearrange("(p j) s -> p j s", j=NJ) for b in range(B2)]
    attn_v = [attn_d[b].rearrange("(p j) s -> p j s", j=NJ) for b in range(B2)]
    alp_v = [alp_d[b].rearrange("(p j) s -> p j s", j=NJ) for b in range(B2)]

    with tile.TileContext(nc) as tc:
        with (
            tc.tile_pool(name="consts", bufs=1) as consts,
            tc.tile_pool(name="persist", bufs=1) as persist,
            tc.tile_pool(name="work", bufs=2) as work,
            tc.tile_pool(name="psA", bufs=2, space="PSUM") as psA,
            tc.tile_pool(name="psQ", bufs=5, space="PSUM") as psQ,
            tc.tile_pool(name="psT", bufs=1, space="PSUM") as psT,
        ):
            # ---------------- input loads (sync ring is FIFO: kconv chain first) ----------------
            kspkt = consts.tile([128, 2, 512], BF16)
            nc.sync.dma_start(kspkt, kspkt_d[:])
            gT = consts.tile([128, 2, B2], BF16)
            nc.sync.dma_start(gT, g_d[:])
            bkspk = consts.tile([128, 4], F32)
            nc.sync.dma_start(bkspk, bkspk_d[:])
            bk1 = consts.tile([128, 8], F32)
            nc.sync.dma_start(bk1, bk1_d[:])
            keysf = work.tile([128, 4, B2, T2], BF16, tag="keysf")
            nc.sync.dma_start(keysf, keys_d[:])
            w1t = consts.tile([128, 8, 3, 4, 128], BF16)   # [p, co, k, c, m]
            for co in range(0, 8, 2):
                nc.sync.dma_start(w1t[:, co], w1t_d[co])
            w2t = consts.tile([128, 8, 80], BF16)
            nc.sync.dma_start(w2t, w2t_d[:])
            bk2 = consts.tile([80, 1], F32)
            nc.sync.dma_start(bk2, bk2_d[:])

            # ---------------- q-chain loads (scalar ring: smalls first) ----------------
            q1t = consts.tile([80, 3, 160], BF16)
            nc.scalar.dma_start(q1t, q1t_d[:])
            q2t = consts.tile([80, 2, 80], BF16)
            nc.scalar.dma_start(q2t, q2t_d[:])
            q3t = consts.tile([80, 80], BF16)
            nc.scalar.dma_start(q3t, q3t_d[:])
            qspkt = consts.tile([128, 2, 80], BF16)
            nc.scalar.dma_start(qspkt, qspkt_d[:])
            bq1 = consts.tile([80, 2], F32)
            nc.scalar.dma_start(bq1, bq1_d[:])
            bq2 = consts.tile([80, 1], F32)
            nc.scalar.dma_start(bq2, bq2_d[:])
            bq3 = consts.tile([80, 1], F32)
            nc.scalar.dma_start(bq3, bq3_d[:])
            bqspk = consts.tile([80, 1], F32)
            nc.scalar.dma_start(bqspk, bqspk_d[:])
            qf = work.tile([80, B2, T1], BF16, tag="qf")
            nc.scalar.dma_start(qf, qry_d[:])
            for co in range(1, 8, 2):
                nc.scalar.dma_start(w1t[:, co], w1t_d[co])

            # ---------------- priors (gpsimd ring) + logpri (ACT: Ln) ----------------
            eps_b = consts.tile([128, 1], F32)
            nc.gpsimd.memset(eps_b, 1e-8)
            ones80 = consts.tile([80, 1], F32)
            nc.gpsimd.memset(ones80, 1.0)
            pri_sb = []
            logpri = []
            for b in range(B2):
                pt = work.tile([TP, NJ, T2], BF16, tag="pri", name=f"pri{b}")
                nc.gpsimd.dma_start(pt, pri_v[b])
                pri_sb.append(pt)
            for b in range(B2):
                lg = persist.tile([TP, NJ, T2], F32, name=f"logpri{b}")
                nc.scalar.activation(lg, pri_sb[b], AF.Ln,
                                     bias=eps_b[:TP, 0:1])
                logpri.append(lg)

            # ---------------- speaker projections (evac on DVE) ----------------
            kspk = persist.tile([128, 4, B2], F32)
            for c in range(4):
                ps = psT.tile([128, B2], F32, tag="tiny")
                for o in range(2):
                    nc.tensor.matmul(
                        ps, kspkt[:, o, c * 128:(c + 1) * 128], gT[:, o, :],
                        start=(o == 0), stop=(o == 1),
                    )
                nc.vector.tensor_scalar_add(kspk[:, c, :], ps, bkspk[:, c:c + 1])
            qspk = persist.tile([80, B2], F32)
            psq = psT.tile([80, B2], F32, tag="tiny")
            for o in range(2):
                nc.tensor.matmul(psq, qspkt[:, o, :], gT[:, o, :],
                                 start=(o == 0), stop=(o == 1))
            nc.vector.tensor_scalar_add(qspk, psq, bqspk[:, 0:1])

            # ---------------- conditioned + padded inputs ----------------
            keys_bf = persist.tile([128, 4, B2, T2 + 2], BF16)
            nc.gpsimd.memset(keys_bf[:, :, :, 0:1], 0.0)
            nc.gpsimd.memset(keys_bf[:, :, :, T2 + 1:T2 + 2], 0.0)
            for c in range(4):
                for b in range(B2):
                    nc.vector.tensor_scalar_add(
                        keys_bf[:, c, b, 1:T2 + 1], keysf[:, c, b, :],
                        kspk[:, c, b:b + 1],
                    )
            q_bf = persist.tile([80, B2, T1 + 2], BF16)
            nc.gpsimd.memset(q_bf[:, :, 0:1], 0.0)
            nc.gpsimd.memset(q_bf[:, :, T1 + 1:T1 + 2], 0.0)
            for b in range(B2):
                nc.vector.tensor_scalar_add(
                    q_bf[:, b, 1:T1 + 1], qf[:, b, :], qspk[:, b:b + 1]
                )

            # ---------------- query convs first: PE work that needs no w1t ----------------
            x1 = persist.tile([80, 2, B2, 2, 500], BF16)  # [p, m, b, half, t]
            for m in range(2):
                for b in range(B2):
                    for j in range(2):
                        ps = psQ.tile([80, 500], F32, tag="q")
                        for k in range(3):
                            nc.tensor.matmul(
                                ps,
                                q1t[:, k, m * 80:(m + 1) * 80],
                                q_bf[:, b, j * 500 + k:j * 500 + k + 500],
                                start=(k == 0), stop=(k == 2),
                            )
                        nc.scalar.activation(x1[:, m, b, j], ps, AF.Relu,
                                             bias=bq1[:, m:m + 1])
            x2q = persist.tile([80, B2, 2, 500], BF16)
            for b in range(B2):
                for j in range(2):
                    ps = psQ.tile([80, 500], F32, tag="q")
                    for ci in range(2):
                        nc.tensor.matmul(ps, q2t[:, ci, :], x1[:, ci, b, j],
                                         start=(ci == 0), stop=(ci == 1))
                    nc.scalar.activation(x2q[:, b, j], ps, AF.Relu,
                                         bias=bq2[:, 0:1])
            q_aug = persist.tile([97, B2, T1], BF16)
            nc.gpsimd.memset(q_aug, 0.0)
            nc.gpsimd.memset(q_aug[96:97], 1.0)
            for b in range(B2):
                for j in range(2):
                    ps = psQ.tile([80, 500], F32, tag="q")
                    nc.tensor.matmul(ps, q3t, x2q[:, b, j], start=True, stop=True)
                    nc.vector.tensor_scalar_add(
                        q_aug[0:80, b, j * 500:(j + 1) * 500], ps, bq3[:, 0:1])
            # [p(contraction), b, j, tp] view: column index tp maps to t = 8*tp + j
            q_aug_v = q_aug.rearrange("p b (tp j) -> p b j tp", j=NJ)

            # ---------------- key conv1 (k=3) + relu (ACT) -> x2, kconv2 interleaved ----------------
            x2 = persist.tile([128, 8, B2, T2], BF16)
            rhs_sc = persist.tile([97, B2, T2], BF16)
            nc.gpsimd.memset(rhs_sc, 0.0)
            ps2 = psQ.tile([80, B2, T2], F32, tag="q")
            for co in range(8):
                ps = psA.tile([128, B2, T2], F32, tag="kc1")
                for k in range(3):
                    for c in range(4):
                        nc.tensor.matmul(
                            ps,
                            w1t[:, co, k, c, :],
                            keys_bf[:, c, :, k:k + T2],
                            start=(c == 0 and k == 0),
                            stop=(c == 3 and k == 2),
                        )
                nc.scalar.activation(x2[:, co], ps, AF.Relu, bias=bk1[:, co:co + 1])
                nc.tensor.matmul(ps2, w2t[:, co, :], x2[:, co],
                                 start=(co == 0), stop=(co == 7))
            # k_enc rows (bf16) and |k|^2 row, straight from the conv2 PSUM
            nc.vector.tensor_scalar_add(rhs_sc[0:80], ps2, bk2[:, 0:1])
            ksq = work.tile([80, B2, T2], F32, tag="ksq")
            nc.scalar.activation(ksq, ps2, AF.Square, bias=bk2[:, 0:1])
            ps3 = psT.tile([1, B2, T2], F32, tag="tiny")
            nc.tensor.matmul(ps3, ones80, ksq, start=True, stop=True)
            nc.vector.tensor_scalar_mul(rhs_sc[96:97], ps3, -TEMP)

            # ---------------- mask replication (generic path only) ----------------
            if use_mask:
                mask_sb = consts.tile([1, B2, T2], I32)
                nc.sync.dma_start(
                    mask_sb,
                    mask_d.rearrange("b t -> (b t)")[None, :]
                          .rearrange("o (b t) -> o b t", b=B2))
                m01 = consts.tile([1, B2, T2], F32)
                nc.vector.tensor_scalar(m01, mask_sb, 0, None, ALU.not_equal)
                ones1 = consts.tile([1, 128], F32)
                nc.vector.memset(ones1, 1.0)
                psm = psT.tile([128, B2, T2], F32, tag="tiny")
                for b in range(B2):
                    nc.tensor.matmul(psm[:, b, :], ones1, m01[:, b, :],
                                     start=True, stop=True)
                mrep = persist.tile([128, B2, T2], F32)
                nc.vector.tensor_copy(mrep, psm)

            # ---------------- scores + softmax, pipelined per batch ----------------
            # attn = normalize_s(exp(lp_pre)): the -lse row shift cancels in
            # the softmax, so the attn branch never waits on logsumexp.
            NP = NJ // 2  # j-pairs (each pair shares one PSUM bank)
            rsums = persist.tile([TP, B2, NJ], F32)
            rinvs = persist.tile([TP, B2, NJ], F32)
            nlse = persist.tile([TP, B2, NJ], F32)
            fsums = persist.tile([TP, B2, NJ], F32)
            frs = persist.tile([TP, B2, NJ], F32)
            out_eng = [nc.sync, nc.gpsimd]
            e_sb = []
            lp_sb = []
            for b in range(B2):
                e_m = persist.tile([TP, NJ, T2], F32, name=f"e{b}")
                lp_m = persist.tile([TP, NJ, T2], F32, name=f"lp{b}")
                e_sb.append(e_m)
                lp_sb.append(lp_m)
                for jp in range(NP):
                    ps = psQ.tile([TP, 2, T2], F32, tag="q")
                    for u in range(2):
                        j = jp * 2 + u
                        nc.tensor.matmul(ps[:, u, :], q_aug_v[:, b, j, :],
                                         rhs_sc[:, b, :], start=True, stop=True)
                    pr = slice(jp * 2, jp * 2 + 2)
                    nc.scalar.activation(e_m[:, pr, :], ps, AF.Exp)
                    for u in range(2):
                        j = jp * 2 + u
                        nc.vector.tensor_tensor(
                            lp_m[:, j, :], ps[:, u, :], logpri[b][:, j, :],
                            ALU.add)
                    nc.vector.reduce_sum(rsums[:, b, pr], e_m[:, pr, :],
                                         axis=mybir.AxisListType.X)
                    if use_mask:
                        # f = exp(lp_pre) * mask, fsum via DVE
                        nc.scalar.activation(e_m[:, pr, :], lp_m[:, pr, :],
                                             AF.Exp)
                        nc.vector.tensor_tensor(
                            e_m[:, pr, :], e_m[:, pr, :],
                            mrep[:TP, b, None, :].to_broadcast([TP, 2, T2]),
                            ALU.mult)
                        nc.vector.reduce_sum(fsums[:, b, pr], e_m[:, pr, :],
                                             axis=mybir.AxisListType.X)
                    else:
                        nc.scalar.activation(e_m[:, pr, :], lp_m[:, pr, :],
                                             AF.Exp)
                        nc.vector.reduce_sum(fsums[:, b, pr], e_m[:, pr, :],
                                             axis=mybir.AxisListType.X)
                # attn = f / fsum (bf16 out), out as soon as each pair is scaled
                at_m = persist.tile([TP, NJ, T2], BF16, name=f"at{b}")
                nc.vector.reciprocal(frs[:, b, :], fsums[:, b, :])
                for jp in range(NP):
                    pr = slice(jp * 2, jp * 2 + 2)
                    for u in range(2):
                        j = jp * 2 + u
                        nc.vector.tensor_scalar_mul(
                            at_m[:, j, :], e_m[:, j, :], frs[:, b, j:j + 1])
                    out_eng[jp % 2].dma_start(attn_v[b][:, pr, :],
                                              at_m[:, pr, :])

            # attn_logprob needs nlse = ln(1/rowsum): one Ln for all rows
            nc.vector.reciprocal(rinvs, rsums)
            nc.scalar.activation(nlse, rinvs, AF.Ln)
            for b in range(B2):
                lo_m = persist.tile([TP, NJ, T2], BF16, name=f"lo{b}")
                for jp in range(NP):
                    pr = slice(jp * 2, jp * 2 + 2)
                    for u in range(2):
                        j = jp * 2 + u
                        nc.vector.tensor_scalar_add(
                            lo_m[:, j, :], lp_sb[b][:, j, :],
                            nlse[:, b, j:j + 1])
                    out_eng[(jp + 1) % 2].dma_start(alp_v[b][:, pr, :],
                                                    lo_m[:, pr, :])

    nc.compile()
    return nc


_CACHE: dict = {}


def _prep_shared(wk1, bk1, wk2, bk2, wq1, bq1, wq2, bq2, wq3, bq3,
                 w_kspk, b_kspk, w_qspk, b_qspk):
    bf = ml_dtypes.bfloat16
    f32 = np.float32

    def a(x):
        return np.ascontiguousarray(np.asarray(x, f32))

    def c(x):
        return np.ascontiguousarray(x)

    shared = {
        # [p(cin%128), k, cin//128, cout]
        # [co, p(cin%128), k, cin//128, m(cout%128)]
        "w1t": c(np.transpose(a(wk1), (2, 1, 0)).reshape(3, 4, 128, 8, 128)
                 .transpose(3, 2, 0, 1, 4)).astype(bf),
        "w2t": c(a(wk2)[:, :, 0].T.reshape(8, 128, 80)
                 .transpose(1, 0, 2)).astype(bf),
        # [cin(p)=80, k, cout=160]; wq1 is [160, 80, 3]
        "q1t": c(np.transpose(a(wq1), (1, 2, 0))).astype(bf),
        "q2t": c(a(wq2)[:, :, 0].T.reshape(2, 80, 80)
                 .transpose(1, 0, 2)).astype(bf),
        "q3t": c(2.0 * TEMP * a(wq3)[:, :, 0].T).astype(bf),
        "kspkt": c(a(w_kspk).T.reshape(2, 128, 512)
                   .transpose(1, 0, 2)).astype(bf),
        "qspkt": c(a(w_qspk).T.reshape(2, 128, 80)
                   .transpose(1, 0, 2)).astype(bf),
        "bk1": c(a(bk1).reshape(8, 128).T),
        "bk2": a(bk2).reshape(80, 1),
        "bq1": c(a(bq1).reshape(2, 80).T),
        "bq2": a(bq2).reshape(80, 1),
        "bq3": (2.0 * TEMP * a(bq3)).reshape(80, 1),
        "bkspk": c(a(b_kspk).reshape(4, 128).T),
        "bqspk": a(b_qspk).reshape(80, 1),
    }
    return shared




def make_in_maps(queries, keys, mask, attn_prior, g,
                 wk1, bk1, wk2, bk2, wq1, bq1, wq2, bq2, wq3, bq3,
                 w_kspk, b_kspk, w_qspk, b_qspk,
                 n_cores=8, use_mask=False):
    shared = _prep_shared(wk1, bk1, wk2, bk2, wq1, bq1, wq2, bq2, wq3, bq3,
                          w_kspk, b_kspk, w_qspk, b_qspk)
    queries = np.asarray(queries, np.float32)
    keys = np.asarray(keys, np.float32)
    attn_prior = np.asarray(attn_prior, np.float32)
    g = np.asarray(g, np.float32)
    mask = np.ascontiguousarray(np.asarray(mask, np.int32))
    in_maps = []
    for ci in range(n_cores):
        sl = slice(B2 * ci, B2 * (ci + 1))
        m = dict(shared)
        m["keys"] = np.ascontiguousarray(
            keys[sl].reshape(B2, 4, 128, T2).transpose(2, 1, 0, 3)
        ).astype(ml_dtypes.bfloat16)
        m["queries"] = np.ascontiguousarray(queries[sl].transpose(1, 0, 2)).astype(ml_dtypes.bfloat16)
        m["prior"] = np.ascontiguousarray(attn_prior[sl]).astype(ml_dtypes.bfloat16)
        m["g"] = np.ascontiguousarray(
            np.transpose(g[sl].reshape(B2, 2, 128), (2, 1, 0))
        ).astype(ml_dtypes.bfloat16)
        if use_mask:
            m["mask"] = mask[sl]
        in_maps.append(m)
    return in_maps

def kernel(queries, keys, mask, attn_prior, g,
           wk1, bk1, wk2, bk2, wq1, bq1, wq2, bq2, wq3, bq3,
           w_kspk, b_kspk, w_qspk, b_qspk,
           _trace=False, _trace_kwargs=None):
    n_cores = 8
    B = 16
    use_mask = not bool(np.all(np.asarray(mask) != 0))

    key = ("nc", use_mask)
    if key not in _CACHE:
        _CACHE[key] = build_nc(use_mask)
    nc = _CACHE[key]

    in_maps = make_in_maps(
        queries, keys, mask, attn_prior, g,
        wk1, bk1, wk2, bk2, wq1, bq1, wq2, bq2, wq3, bq3,
        w_kspk, b_kspk, w_qspk, b_qspk,
        n_cores=n_cores, use_mask=use_mask)

    kw = {}
    if _trace:
        kw["trace"] = True
        kw.update(_trace_kwargs or {})
    res = run_bass_kernel_spmd(nc, in_maps, list(range(n_cores)), **kw)

    attn = np.concatenate([r["attn"] for r in res.results], axis=0)
    alp = np.concatenate([r["alp"] for r in res.results], axis=0)
    attn = attn.reshape(B, 1, T1, T2).astype(np.float32)
    alp = alp.reshape(B, 1, T1, T2).astype(np.float32)
    if _trace:
        return (attn, alp), res
    return attn, alp
